# revision 19
# baseline (speedup 1.0000x reference)
"""Distributed multi-head attention kernel for 8 TRN2 NeuronCores.

Problem: x(4,2048,1024) -> qkv proj (w_qkv 3072x1024) -> 16-head attention
(head_dim 64, softmax) -> out proj (w_out 1024x1024 + b_out).

Sharding: head-parallel. Core c owns heads {2c, 2c+1}; per-batch AllToAll
(bf16) converts head-sharded attention output to token-sharded layout for
the output projection (no all-reduce needed).

Structure:
- S matmuls for the two heads are emitted back-to-back and pack into PE row
  groups 0-63 / 64-127 (tile_position derived from base partitions), running
  concurrently (pair members retire ~4ns apart).
- PV keeps a 65th ones-column in V so the softmax denominators accumulate
  for free in PSUM: every alternative denominator costs more than PV
  col-packing would save (measured DVE tensor ops run at only ~100-180G
  elem/s, and a ones-matmul partition reduce costs the same 512 cycles as
  the PV matmul itself).
- Score PSUM: alternating quad [128,2048] / duo [128,1024] tiles (4+2
  banks, ring=1 per tag) + 2 PV banks [128,512] = 8 banks. exp runs
  2048/1024-wide on ScalarE (the ~250us floor engine: 34M exp elements at
  1 elem/lane/cycle @1.2GHz). PV flushes trail the score tiles by two slots
  so the in-order PE queue never head-stalls on a fresh exp.
- QKV for batch b+1 (one 512-token tile per qtile) and out-proj for batch
  b-1 (one m-tile burst per slot) are woven between score tiles so ScalarE
  never starves at batch boundaries and the PE stays dense (HAM clock).
- Batch 3's AllToAll is split into three pieces (qt0+1, qt2, qt3) so the
  tail exposes only a 64-token collective + small out-proj; a dummy
  collective and a dummy exp at t=0 absorb the CC cold-start and the
  ACT_TABLE_LOAD.

Measured on 8 axon-tunneled trn2 cores: ~490us HW exec, rel err 5.2e-3.
(PE is the binding engine at ~395us busy: the chip sustains ~2.0GHz under
8-core load, so QKV 100us + S-pairs 94us + PV 155us + V 33us + out-proj
~45us; ScalarE exp ~254us overlaps underneath.)
"""

import numpy as np
import ml_dtypes

import concourse.bass as bass
import concourse.mybir as mybir
import concourse.tile as tile
from concourse import bacc, bass_utils
from concourse.tile import add_dep_helper

FP32 = mybir.dt.float32
BF16 = mybir.dt.bfloat16
AF = mybir.ActivationFunctionType

N_CORES = 8
B, NTOK, D = 4, 2048, 1024
T = B * NTOK  # 8192 tokens total
NH, HD = 16, 64
HL = NH // N_CORES  # 2 heads per core
SCALE = float(HD) ** -0.5  # 0.125
TN = 512  # q tile width
KT = NTOK // 128  # 16 k-chunks per batch
KC = D // 128  # 8 contraction chunks for projections
TPB = NTOK // N_CORES  # 256 tokens per (core, batch) after A2A
TPC = T // N_CORES  # 1024 tokens per core total
WCOLS = 3 * HL * HD  # 384 qkv output dims per core


def build_nc():
    nc = bacc.Bacc(
        "TRN2", target_bir_lowering=False, debug=False, num_devices=N_CORES
    )
    xt = nc.dram_tensor("xt", [D, T], BF16, kind="ExternalInput").ap()
    wt = nc.dram_tensor("wt", [D, WCOLS], BF16, kind="ExternalInput").ap()
    wo = nc.dram_tensor("wo", [D, D], BF16, kind="ExternalInput").ap()
    bias = nc.dram_tensor("bias", [1, D], FP32, kind="ExternalInput").ap()
    out = nc.dram_tensor("out", [TPC, D], FP32, kind="ExternalOutput").ap()

    with tile.TileContext(nc) as tc:
        with (
            tc.tile_pool(name="const", bufs=1) as const,
            tc.tile_pool(name="xin", bufs=3) as xin,
            tc.tile_pool(name="pq", bufs=4) as pqp,
            tc.tile_pool(name="pd", bufs=4) as pdp,
            tc.tile_pool(name="norm", bufs=4) as norm,
            tc.tile_pool(name="ot", bufs=4) as otp,
            tc.tile_pool(name="osb", bufs=2) as osbp,
            tc.tile_pool(name="fin", bufs=2) as fin,
            tc.tile_pool(name="psum", bufs=1, space="PSUM") as psum,
            tc.tile_pool(name="dram", bufs=1, space="DRAM") as dram,
        ):
            # ---- persistent SBUF state ----
            # w_sb loads first (QKV t0 needs it immediately); wo_sb/bias are
            # not needed until the first out-proj (~batch 1 qt2), so their
            # loads are deferred below the first x-tile loads.
            w_sb = const.tile([128, KC * WCOLS], BF16)
            nc.sync.dma_start(
                w_sb[:].rearrange("p (kc j) -> p kc j", kc=KC),
                wt.rearrange("(kc p) j -> p kc j", p=128),
            )
            # dummy exp at t=0 so the ~2.7us ACT_TABLE_LOAD overlaps the
            # initial DMAs instead of delaying the first real exp
            warm = const.tile([1, 2], FP32)
            nc.vector.memset(warm[:], 0.0)
            nc.scalar.activation(warm[:], warm[:], AF.Exp)
            # dummy collective at t~0: absorbs the cold-start cost of the CC
            # path (~20us) so batch 0's real AllToAll runs warm
            warm_in = dram.tile([N_CORES, 1, 64], BF16, name="warm_in")
            warm_out = dram.tile([N_CORES, 1, 64], BF16, name="warm_out")
            wz = const.tile([1, N_CORES * 64], BF16)
            nc.vector.memset(wz[:], 0.0)
            nc.sync.dma_start(
                warm_in[:, :, :].rearrange("i p e -> p i e"),
                wz[:].rearrange("p (i e) -> p i e", i=N_CORES),
            )
            wo_sb = const.tile([128, KC * D], BF16)
            b_row = const.tile([1, D], FP32)
            bias_sb = const.tile([128, D], FP32)

            def emit_wo_load():
                nc.sync.dma_start(
                    wo_sb[:].rearrange("p (kc n) -> p kc n", kc=KC),
                    wo.rearrange("(kc p) n -> p kc n", p=128),
                )
                nc.sync.dma_start(b_row[:], bias[:])
                nc.gpsimd.partition_broadcast(bias_sb[:], b_row[:])

            q_sb = const.tile([128, T], BF16)  # [2 heads x 64, tokens], scaled
            k_sb = const.tile([128, T], BF16)
            # V token-major with ones column: [128, blk=(chunk,head), 65]
            v_sb = const.tile([128, (T // 128) * HL * 65], BF16)
            v3 = v_sb[:].rearrange("p (blk e) -> p blk e", e=65)
            nc.vector.memset(v3[:, :, 64:65], 1.0)

            a2a_in = {}
            a2a_out = {}
            for b in range(B - 1):
                a2a_in[b] = dram.tile(
                    [N_CORES, HL * HD, TPB], BF16, name=f"a2a_in{b}"
                )
                a2a_out[b] = dram.tile(
                    [N_CORES, HL * HD, TPB], BF16, name=f"a2a_out{b}"
                )
            # last batch: 3 pieces (qt0+qt1: 128 tok/core; qt2: 64; qt3: 64)
            a2a_in3 = {}
            a2a_out3 = {}
            for pi, wtok in ((0, 128), (1, 64), (2, 64)):
                a2a_in3[pi] = dram.tile(
                    [N_CORES, HL * HD, wtok], BF16, name=f"a2a_in3_{pi}"
                )
                a2a_out3[pi] = dram.tile(
                    [N_CORES, HL * HD, wtok], BF16, name=f"a2a_out3_{pi}"
                )

            def emit_a2a(ins, outs):
                nc.gpsimd.collective_compute(
                    "AllToAll",
                    mybir.AluOpType.bypass,
                    replica_groups=[list(range(N_CORES))],
                    ins=[ins.opt()],
                    outs=[outs.opt()],
                )

            emit_a2a(warm_in, warm_out)  # CC cold-start warmup

            # ---------------- x prefetch ----------------
            x_tiles = {}  # global 512-token tile idx -> list of 8 chunk tiles

            def emit_x_load(t):
                x_t = xin.tile([128, KC * TN], BF16, tag="xt", name="x_t")
                nc.sync.dma_start(
                    x_t[:].rearrange("p (kc e) -> p kc e", kc=KC),
                    xt[:, t * TN : (t + 1) * TN].rearrange(
                        "(kc p) e -> p kc e", p=128
                    ),
                )
                x_tiles[t] = x_t

            # ---------------- QKV projection, one 512-token tile ----------------
            def emit_qkv_tile(t):
                y = psum.tile([128, 2048], FP32, tag="q4", name="y_ps")
                xts = x_tiles.pop(t)
                for kc in range(KC):
                    st, sp = kc == 0, kc == KC - 1
                    for m in range(2):  # Q then K, transposed layout
                        nc.tensor.matmul(
                            y[:, m * 512 : (m + 1) * 512],
                            lhsT=w_sb[
                                :,
                                kc * WCOLS + m * 128 : kc * WCOLS + (m + 1) * 128,
                            ],
                            rhs=xts[:, kc * TN : (kc + 1) * TN],
                            start=st,
                            stop=sp,
                        )
                # V natural layout: 4 token subtiles share one PSUM bank;
                # start=True clears has_written flags bank-wide, so chain
                # ordering deps so each accumulation group finishes first.
                prev = None
                for s in range(4):
                    for kc in range(KC):
                        mm = nc.tensor.matmul(
                            y[:, 1024 + s * 128 : 1024 + (s + 1) * 128],
                            lhsT=xts[
                                :, kc * TN + s * 128 : kc * TN + (s + 1) * 128
                            ],
                            rhs=w_sb[:, kc * WCOLS + 256 : kc * WCOLS + WCOLS],
                            start=(kc == 0),
                            stop=(kc == KC - 1),
                        )
                        if prev is not None:
                            add_dep_helper(
                                mm.ins, prev.ins, sync=False,
                                reason="bank flag-clear order",
                            )
                        prev = mm
                # epilogues on VectorE (keep ScalarE free for exp)
                nc.vector.tensor_scalar_mul(
                    q_sb[:, t * TN : (t + 1) * TN], y[:, 0:512], SCALE
                )
                nc.vector.tensor_copy(
                    k_sb[:, t * TN : (t + 1) * TN], y[:, 512:1024]
                )
                nc.vector.tensor_copy(
                    v3[:, (t * 4) * HL : (t * 4 + 4) * HL, 0:64],
                    y[:, 1024:1536]
                    .rearrange("p (s hd) -> p s hd", s=4)
                    .rearrange("p s (h d) -> p (s h) d", h=HL),
                )

            # ---------------- per-qt attention state ----------------
            class QtState:
                def __init__(self, b, qt):
                    self.b = b
                    self.qt = qt
                    self.pv = None
                    self.p_slots = {}  # kc -> (p_tile, col offset of h0)

            def flush_pv(st_, kc):
                if st_.pv is None:
                    st_.pv = [
                        psum.tile([128, 512], FP32, tag=f"pv{h}", name=f"pv{h}")
                        for h in range(HL)
                    ]
                p_t, off = st_.p_slots.pop(kc)
                first, last = kc == 0, kc == KT - 1
                gc = st_.b * KT + kc
                for h in range(HL):
                    nc.tensor.matmul(
                        st_.pv[h][0:65, :],
                        lhsT=v3[:, gc * HL + h, :],
                        rhs=p_t[:, off + h * 512 : off + (h + 1) * 512],
                        start=first,
                        stop=last,
                    )

            def finish_qt(st_):
                b, qt = st_.b, st_.qt
                for h in range(HL):
                    # single copy releases the PV PSUM bank; the rest of the
                    # normalize chain runs on SBUF off the fast path
                    o_c = norm.tile([65, 512], FP32, tag="oc", name="o_c")
                    nc.vector.tensor_copy(o_c[:], st_.pv[h][0:65, :])
                    # reciprocal across 128 partitions (not 512 sequential)
                    rs = norm.tile([128, 4], FP32, tag="rs", name="rs")
                    nc.sync.dma_start(rs[:], o_c[64:65, :])
                    rr = norm.tile([128, 4], FP32, tag="rr", name="rr")
                    nc.vector.reciprocal(rr[:], rs[:])
                    rec = norm.tile([1, 512], FP32, tag="rec", name="rec")
                    nc.sync.dma_start(rec[:], rr[:])
                    bc = norm.tile([64, 512], FP32, tag="bc", name="bc")
                    nc.gpsimd.partition_broadcast(bc[:], rec[:])
                    o_t = otp.tile([64, 512], BF16, tag="o", name="o_t")
                    nc.vector.tensor_mul(o_t[:], o_c[0:64, :], bc[:])
                    if b < B - 1:
                        nc.sync.dma_start(
                            a2a_in[b][
                                2 * qt : 2 * qt + 2, h * 64 : (h + 1) * 64, :
                            ].rearrange("j p e -> p j e"),
                            o_t[:].rearrange("p (j e) -> p j e", j=2),
                        )
                    elif qt < 2:
                        j0 = (qt % 2) * 4
                        nc.sync.dma_start(
                            a2a_in3[0][
                                j0 : j0 + 4, h * 64 : (h + 1) * 64, :
                            ].rearrange("j p e -> p j e"),
                            o_t[:].rearrange("p (j e) -> p j e", j=4),
                        )
                    else:
                        nc.sync.dma_start(
                            a2a_in3[qt - 1][
                                :, h * 64 : (h + 1) * 64, :
                            ].rearrange("j p e -> p j e"),
                            o_t[:].rearrange("p (j e) -> p j e", j=8),
                        )

            # ---------------- score tiles ----------------
            def emit_stile(st_, kcs):
                b, qt = st_.b, st_.qt
                q_off = b * NTOK + qt * TN
                if len(kcs) == 2:
                    s_t = psum.tile([128, 2048], FP32, tag="q4", name="s_q")
                    p_t = pqp.tile([128, 2048], BF16, tag="p4", name="p_q")
                else:
                    s_t = psum.tile([128, 1024], FP32, tag="d2", name="s_d")
                    p_t = pdp.tile([128, 1024], BF16, tag="p2", name="p_d")
                width = 1024 * len(kcs)
                for ci, kc in enumerate(kcs):
                    koff = b * NTOK + kc * 128
                    for h in range(HL):
                        nc.tensor.matmul(
                            s_t[:, (ci * 2 + h) * 512 : (ci * 2 + h + 1) * 512],
                            lhsT=k_sb[h * 64 : (h + 1) * 64, koff : koff + 128],
                            rhs=q_sb[h * 64 : (h + 1) * 64, q_off : q_off + TN],
                            start=True,
                            stop=True,
                        )
                nc.scalar.activation(p_t[:, 0:width], s_t[:, 0:width], AF.Exp)
                for ci, kc in enumerate(kcs):
                    st_.p_slots[kc] = (p_t, ci * 1024)

            # ---------------- out projection ----------------
            oproj_state = {}

            def emit_osb_load(bsrc):
                o_sb = osbp.tile(
                    [128, N_CORES * TPB], BF16, tag="osb", name="o_sb"
                )
                nc.sync.dma_start(
                    o_sb[:].rearrange("p (i e) -> p i e", i=N_CORES),
                    a2a_out[bsrc][:, :, :].rearrange("i p e -> p i e"),
                )
                oproj_state["sb"] = o_sb

            def emit_outproj_m(bsrc, m):
                o_sb = oproj_state["sb"]
                o_ps = psum.tile([128, 1024], FP32, tag="d2", name="o_ps")
                for i in range(N_CORES):
                    for nh in range(2):
                        nc.tensor.matmul(
                            o_ps[:, nh * 512 : (nh + 1) * 512],
                            lhsT=o_sb[
                                :, i * TPB + m * 128 : i * TPB + (m + 1) * 128
                            ],
                            rhs=wo_sb[
                                :, i * D + nh * 512 : i * D + nh * 512 + 512
                            ],
                            start=(i == 0),
                            stop=(i == N_CORES - 1),
                        )
                out_t = fin.tile([128, D], FP32, tag="outt", name="out_t")
                nc.vector.tensor_add(out_t[:, :], o_ps[:, :], bias_sb[:, :])
                nc.sync.dma_start(
                    out[bsrc * TPB + m * 128 : bsrc * TPB + (m + 1) * 128, :],
                    out_t[:],
                )

            def emit_outproj3(pi, o_sbh, wtok, row0):
                # piece pi of batch 3: wtok tokens/core starting at out row0
                o_ps = psum.tile([128, 1024], FP32, tag="d2", name="o3_ps")
                for i in range(N_CORES):
                    for nh in range(2):
                        nc.tensor.matmul(
                            o_ps[0:wtok, nh * 512 : (nh + 1) * 512],
                            lhsT=o_sbh[:, i * wtok : (i + 1) * wtok],
                            rhs=wo_sb[
                                :, i * D + nh * 512 : i * D + nh * 512 + 512
                            ],
                            start=(i == 0),
                            stop=(i == N_CORES - 1),
                        )
                out_t = fin.tile([128, D], FP32, tag="outt", name="out_t")
                nc.vector.tensor_add(
                    out_t[0:wtok, :], o_ps[0:wtok, :], bias_sb[0:wtok, :]
                )
                nc.sync.dma_start(
                    out[row0 : row0 + wtok, :], out_t[0:wtok, :]
                )

            def emit_osbh_load(pi, wtok):
                o_sbh = osbp.tile(
                    [128, N_CORES * 128], BF16, tag="osbh", name=f"o_sbh{pi}"
                )
                nc.sync.dma_start(
                    o_sbh[:, 0 : N_CORES * wtok].rearrange(
                        "p (i e) -> p i e", i=N_CORES
                    ),
                    a2a_out3[pi][:, :, :].rearrange("i p e -> p i e"),
                )
                return o_sbh

            # ---------------- schedule ----------------
            def qt_stiles():
                # [Q(0,1) D(2) Q(3,4) D(5) Q(6,7) D(8) Q(9,10) D(11)
                #  Q(12,13) D(14) D(15)]
                s, kc = [], 0
                for _ in range(5):
                    s.append([kc, kc + 1])
                    kc += 2
                    s.append([kc])
                    kc += 1
                s.append([kc])
                return s

            pending = []  # (state, kcs) score tiles with PV flush deferred

            def note_stile(st_, kcs):
                # flush PV (three score-tiles behind) BEFORE the new stile's
                # S matmuls: the ready PV work then precedes the S quad in
                # the in-order PE FIFO, so a quad waiting on its ring's exp
                # no longer head-blocks work that could run
                while len(pending) > 2:
                    pst, pkcs = pending.pop(0)
                    for kc in pkcs:
                        flush_pv(pst, kc)
                emit_stile(st_, kcs)
                pending.append((st_, list(kcs)))

            def drain_pending():
                while pending:
                    pst, pkcs = pending.pop(0)
                    for kc in pkcs:
                        flush_pv(pst, kc)

            prev_qt_state = None

            # ---- batch 0 prologue: own QKV + qt0 interleaved ----
            emit_x_load(0)
            emit_x_load(1)
            st0 = QtState(0, 0)
            stiles0 = qt_stiles()
            g_next = 0
            for i in range(4):
                emit_qkv_tile(i)
                emit_x_load(i + 2)  # tiles 2..5 (tiles 4,5 belong to b1)
                if i == 0:
                    emit_wo_load()  # deferred so x0/w_sb loads go first
                avail = 4 * (i + 1)
                while g_next < len(stiles0) and all(
                    kc < avail for kc in stiles0[g_next]
                ):
                    note_stile(st0, stiles0[g_next])
                    g_next += 1
            emit_qkv_tile(4)  # b1 tile 0
            prev_qt_state = st0

            for b in range(B):
                for qt in range(4):
                    if b == 0 and qt == 0:
                        continue  # prologue handled it
                    st_ = QtState(b, qt)
                    # prefetch x for the QKV tile one slot ahead
                    nqt = 4 * (b + 1) + qt + 1
                    if nqt < 16 and nqt not in x_tiles:
                        emit_x_load(nqt)
                    for ti, kcs in enumerate(qt_stiles()):
                        # the previous qt's last PV flush happens at ti==2's
                        # note_stile (pending depth 3), so finish at ti==3
                        if ti == 3 and prev_qt_state is not None:
                            finish_qt(prev_qt_state)
                            pb, pq_ = prev_qt_state.b, prev_qt_state.qt
                            prev_qt_state = None
                            if pq_ == 3 and pb < B - 1:
                                emit_a2a(a2a_in[pb], a2a_out[pb])
                            elif pb == B - 1 and pq_ == 1:
                                emit_a2a(a2a_in3[0], a2a_out3[0])
                            elif pb == B - 1 and pq_ == 2:
                                emit_a2a(a2a_in3[1], a2a_out3[1])
                        note_stile(st_, kcs)
                        if ti == 2 and b < B - 1:
                            gt = 4 * (b + 1) + qt
                            if gt in x_tiles:
                                emit_qkv_tile(gt)
                        if b >= 1 and qt == 2 and ti == 6:
                            emit_osb_load(b - 1)
                        if b >= 1 and qt == 2 and ti == 8:
                            emit_outproj_m(b - 1, 0)
                        if b >= 1 and qt == 3 and ti == 4:
                            emit_outproj_m(b - 1, 1)
                    prev_qt_state = st_

            # ---- epilogue: last qt's PV + finish, tail pieces ----
            # piece-0/1 collectives completed during qt3: their out-proj PE
            # work runs under qt3's norm chain + the final collective,
            # keeping HAM warm through the tail
            drain_pending()
            o_sbh0 = emit_osbh_load(0, 128)
            o_sbh1 = emit_osbh_load(1, 64)
            finish_qt(prev_qt_state)  # qt3 -> a2a_in3[2]
            emit_outproj3(0, o_sbh0, 128, 768)
            emit_a2a(a2a_in3[2], a2a_out3[2])
            emit_outproj3(1, o_sbh1, 64, 896)
            o_sbh2 = emit_osbh_load(2, 64)
            emit_outproj3(2, o_sbh2, 64, 960)

    nc.compile()
    return nc


_NC_CACHE = None


def _get_nc():
    global _NC_CACHE
    if _NC_CACHE is None:
        _NC_CACHE = build_nc()
    return _NC_CACHE


def make_in_maps(x, w_qkv, w_out, b_out):
    x = np.asarray(x, dtype=np.float32)
    w_qkv = np.asarray(w_qkv, dtype=np.float32)
    w_out = np.asarray(w_out, dtype=np.float32)
    b_out = np.asarray(b_out, dtype=np.float32)

    xt_np = np.ascontiguousarray(x.reshape(T, D).T).astype(ml_dtypes.bfloat16)
    wo_np = np.ascontiguousarray(w_out.T).astype(ml_dtypes.bfloat16)
    b_np = np.ascontiguousarray(b_out.reshape(1, D))

    in_maps = []
    for c in range(N_CORES):
        rows = []
        for sec in range(3):  # q, k, v sections of w_qkv
            for hh in range(HL):
                h = HL * c + hh
                rows.append(w_qkv[sec * D + h * HD : sec * D + (h + 1) * HD, :])
        wt_np = np.ascontiguousarray(np.concatenate(rows, 0).T).astype(
            ml_dtypes.bfloat16
        )  # (1024, 384)
        in_maps.append({"xt": xt_np, "wt": wt_np, "wo": wo_np, "bias": b_np})
    return in_maps


def kernel(x, w_qkv, w_out, b_out, _trace=False, _tmpdir=None):
    in_maps = make_in_maps(x, w_qkv, w_out, b_out)
    nc = _get_nc()
    res = bass_utils.run_bass_kernel_spmd(
        nc, in_maps, core_ids=list(range(N_CORES)), trace=_trace, tmpdir=_tmpdir
    )
    # core j out rows:
    #   batches 0-2: r = b*256+u       -> token b*2048 + j*256 + u
    #   batch 3 piece0 (qt0+1): r = 768+u  (u<128) -> token 6144 + j*128 + u
    #   batch 3 piece1 (qt2):   r = 896+u  (u<64)  -> token 7168 + j*64 + u
    #   batch 3 piece2 (qt3):   r = 960+u  (u<64)  -> token 7680 + j*64 + u
    full = np.empty((T, D), np.float32)
    for j in range(N_CORES):
        o = np.asarray(res.results[j]["out"], dtype=np.float32)
        for b in range(B - 1):
            full[b * NTOK + j * TPB : b * NTOK + (j + 1) * TPB] = o[
                b * TPB : (b + 1) * TPB
            ]
        full[6144 + j * 128 : 6144 + (j + 1) * 128] = o[768:896]
        full[7168 + j * 64 : 7168 + (j + 1) * 64] = o[896:960]
        full[7680 + j * 64 : 7680 + (j + 1) * 64] = o[960:1024]
    kernel.last_result = res
    return full.reshape(B, NTOK, D)



# revision 22
# speedup vs baseline: 1.1549x; 1.1549x over previous
"""Distributed multi-head attention kernel for 8 TRN2 NeuronCores.

Problem: x(4,2048,1024) -> qkv proj (w_qkv 3072x1024) -> 16-head attention
(head_dim 64, softmax) -> out proj (w_out 1024x1024 + b_out).

Sharding: head-parallel. Core c owns heads {2c, 2c+1}; per-batch AllToAll
(bf16) converts head-sharded attention output to token-sharded layout for
the output projection (no all-reduce needed).

Structure:
- S matmuls for the two heads are emitted back-to-back and pack into PE row
  groups 0-63 / 64-127 (tile_position derived from base partitions), running
  concurrently (pair members retire ~4ns apart).
- PV keeps a 65th ones-column in V so the softmax denominators accumulate
  for free in PSUM: every alternative denominator costs more than PV
  col-packing would save (measured DVE tensor ops run at only ~100-180G
  elem/s, and a ones-matmul partition reduce costs the same 512 cycles as
  the PV matmul itself).
- Score PSUM: alternating quad [128,2048] / duo [128,1024] tiles (4+2
  banks, ring=1 per tag) + 2 PV banks [128,512] = 8 banks. exp runs
  2048/1024-wide on ScalarE (the ~250us floor engine: 34M exp elements at
  1 elem/lane/cycle @1.2GHz). PV flushes trail the score tiles by two slots
  so the in-order PE queue never head-stalls on a fresh exp.
- QKV for batch b+1 (one 512-token tile per qtile) and out-proj for batch
  b-1 (one m-tile burst per slot) are woven between score tiles so ScalarE
  never starves at batch boundaries and the PE stays dense (HAM clock).
- Batch 3's AllToAll is split into three pieces (qt0+1, qt2, qt3) so the
  tail exposes only a 64-token collective + small out-proj; a dummy
  collective and a dummy exp at t=0 absorb the CC cold-start and the
  ACT_TABLE_LOAD.

Measured on 8 axon-tunneled trn2 cores: ~490us HW exec, rel err 5.2e-3.
(PE is the binding engine at ~395us busy: the chip sustains ~2.0GHz under
8-core load, so QKV 100us + S-pairs 94us + PV 155us + V 33us + out-proj
~45us; ScalarE exp ~254us overlaps underneath.)
"""

import numpy as np
import ml_dtypes

import concourse.bass as bass
import concourse.mybir as mybir
import concourse.tile as tile
from concourse import bacc, bass_utils
from concourse.tile import add_dep_helper

FP32 = mybir.dt.float32
BF16 = mybir.dt.bfloat16
AF = mybir.ActivationFunctionType

N_CORES = 8
B, NTOK, D = 4, 2048, 1024
T = B * NTOK  # 8192 tokens total
NH, HD = 16, 64
HL = NH // N_CORES  # 2 heads per core
SCALE = float(HD) ** -0.5  # 0.125
TN = 512  # q tile width
KT = NTOK // 128  # 16 k-chunks per batch
KC = D // 128  # 8 contraction chunks for projections
TPB = NTOK // N_CORES  # 256 tokens per (core, batch) after A2A
TPC = T // N_CORES  # 1024 tokens per core total
WCOLS = 3 * HL * HD  # 384 qkv output dims per core


def build_nc():
    nc = bacc.Bacc(
        "TRN2", target_bir_lowering=False, debug=False, num_devices=N_CORES
    )
    xt = nc.dram_tensor("xt", [D, T], BF16, kind="ExternalInput").ap()
    wt = nc.dram_tensor("wt", [D, WCOLS], BF16, kind="ExternalInput").ap()
    wo = nc.dram_tensor("wo", [D, D], BF16, kind="ExternalInput").ap()
    bias = nc.dram_tensor("bias", [1, D], FP32, kind="ExternalInput").ap()
    out = nc.dram_tensor("out", [TPC, D], FP32, kind="ExternalOutput").ap()

    with tile.TileContext(nc) as tc:
        with (
            tc.tile_pool(name="const", bufs=1) as const,
            tc.tile_pool(name="xin", bufs=18) as xin,
            tc.tile_pool(name="pq", bufs=4) as pqp,
            tc.tile_pool(name="pd", bufs=4) as pdp,
            tc.tile_pool(name="norm", bufs=4) as norm,
            tc.tile_pool(name="ot", bufs=4) as otp,
            tc.tile_pool(name="osb", bufs=2) as osbp,
            tc.tile_pool(name="fin", bufs=2) as fin,
            tc.tile_pool(name="psum", bufs=1, space="PSUM") as psum,
            tc.tile_pool(name="dram", bufs=1, space="DRAM") as dram,
        ):
            # ---- persistent SBUF state ----
            # w_sb loads first (QKV t0 needs it immediately); wo_sb/bias are
            # not needed until the first out-proj (~batch 1 qt2), so their
            # loads are deferred below the first x-tile loads.
            w_sb = const.tile([128, KC * WCOLS], BF16)
            nc.sync.dma_start(
                w_sb[:].rearrange("p (kc j) -> p kc j", kc=KC),
                wt.rearrange("(kc p) j -> p kc j", p=128),
            )
            # dummy exp at t=0 so the ~2.7us ACT_TABLE_LOAD overlaps the
            # initial DMAs instead of delaying the first real exp
            warm = const.tile([1, 2], FP32)
            nc.vector.memset(warm[:], 0.0)
            nc.scalar.activation(warm[:], warm[:], AF.Exp)
            # dummy collective at t~0: absorbs the cold-start cost of the CC
            # path (~20us) so batch 0's real AllToAll runs warm
            warm_in = dram.tile([N_CORES, 1, 64], BF16, name="warm_in")
            warm_out = dram.tile([N_CORES, 1, 64], BF16, name="warm_out")
            wz = const.tile([1, N_CORES * 64], BF16)
            nc.vector.memset(wz[:], 0.0)
            nc.sync.dma_start(
                warm_in[:, :, :].rearrange("i p e -> p i e"),
                wz[:].rearrange("p (i e) -> p i e", i=N_CORES),
            )
            wo_sb = const.tile([128, KC * D], BF16)
            b_row = const.tile([1, D], FP32)
            bias_sb = const.tile([128, D], FP32)

            def emit_wo_load():
                nc.sync.dma_start(
                    wo_sb[:].rearrange("p (kc n) -> p kc n", kc=KC),
                    wo.rearrange("(kc p) n -> p kc n", p=128),
                )
                nc.sync.dma_start(b_row[:], bias[:])
                nc.gpsimd.partition_broadcast(bias_sb[:], b_row[:])

            q_sb = const.tile([128, T], BF16)  # [2 heads x 64, tokens], scaled
            k_sb = const.tile([128, T], BF16)
            # V token-major with ones column: [128, blk=(chunk,head), 65]
            v_sb = const.tile([128, (T // 128) * HL * 65], BF16)
            v3 = v_sb[:].rearrange("p (blk e) -> p blk e", e=65)
            nc.vector.memset(v3[:, :, 64:65], 1.0)

            a2a_in = {}
            a2a_out = {}
            for b in range(B - 1):
                a2a_in[b] = dram.tile(
                    [N_CORES, HL * HD, TPB], BF16, name=f"a2a_in{b}"
                )
                a2a_out[b] = dram.tile(
                    [N_CORES, HL * HD, TPB], BF16, name=f"a2a_out{b}"
                )
            # last batch: 3 pieces (qt0+qt1: 128 tok/core; qt2: 64; qt3: 64)
            a2a_in3 = {}
            a2a_out3 = {}
            for pi, wtok in ((0, 128), (1, 64), (2, 64)):
                a2a_in3[pi] = dram.tile(
                    [N_CORES, HL * HD, wtok], BF16, name=f"a2a_in3_{pi}"
                )
                a2a_out3[pi] = dram.tile(
                    [N_CORES, HL * HD, wtok], BF16, name=f"a2a_out3_{pi}"
                )

            def emit_a2a(ins, outs):
                nc.gpsimd.collective_compute(
                    "AllToAll",
                    mybir.AluOpType.bypass,
                    replica_groups=[list(range(N_CORES))],
                    ins=[ins.opt()],
                    outs=[outs.opt()],
                )

            emit_a2a(warm_in, warm_out)  # CC cold-start warmup

            # ---------------- x prefetch ----------------
            x_tiles = {}  # global 512-token tile idx -> list of 8 chunk tiles

            def emit_x_load(t):
                xts = []
                for kc in range(KC):
                    x_t = xin.tile([128, TN], BF16, tag="xt", name="x_t")
                    nc.sync.dma_start(
                        x_t[:],
                        xt[kc * 128 : (kc + 1) * 128, t * TN : (t + 1) * TN],
                    )
                    xts.append(x_t)
                x_tiles[t] = xts

            # ---------------- QKV projection, one 512-token tile ----------------
            def emit_qkv_tile(t):
                y = psum.tile([128, 2048], FP32, tag="q4", name="y_ps")
                xts = x_tiles.pop(t)
                for kc in range(KC):
                    st, sp = kc == 0, kc == KC - 1
                    for m in range(2):  # Q then K, transposed layout
                        nc.tensor.matmul(
                            y[:, m * 512 : (m + 1) * 512],
                            lhsT=w_sb[
                                :,
                                kc * WCOLS + m * 128 : kc * WCOLS + (m + 1) * 128,
                            ],
                            rhs=xts[kc][:],
                            start=st,
                            stop=sp,
                        )
                # V natural layout: 4 token subtiles share one PSUM bank;
                # start=True clears has_written flags bank-wide, so chain
                # ordering deps so each accumulation group finishes first.
                prev = None
                for s in range(4):
                    for kc in range(KC):
                        mm = nc.tensor.matmul(
                            y[:, 1024 + s * 128 : 1024 + (s + 1) * 128],
                            lhsT=xts[kc][:, s * 128 : (s + 1) * 128],
                            rhs=w_sb[:, kc * WCOLS + 256 : kc * WCOLS + WCOLS],
                            start=(kc == 0),
                            stop=(kc == KC - 1),
                        )
                        if prev is not None:
                            add_dep_helper(
                                mm.ins, prev.ins, sync=False,
                                reason="bank flag-clear order",
                            )
                        prev = mm
                # epilogues on VectorE (keep ScalarE free for exp)
                nc.vector.tensor_scalar_mul(
                    q_sb[:, t * TN : (t + 1) * TN], y[:, 0:512], SCALE
                )
                nc.vector.tensor_copy(
                    k_sb[:, t * TN : (t + 1) * TN], y[:, 512:1024]
                )
                nc.vector.tensor_copy(
                    v3[:, (t * 4) * HL : (t * 4 + 4) * HL, 0:64],
                    y[:, 1024:1536]
                    .rearrange("p (s hd) -> p s hd", s=4)
                    .rearrange("p s (h d) -> p (s h) d", h=HL),
                )

            # ---------------- per-qt attention state ----------------
            class QtState:
                def __init__(self, b, qt):
                    self.b = b
                    self.qt = qt
                    self.pv = None
                    self.p_slots = {}  # kc -> (p_tile, col offset of h0)

            def flush_pv(st_, kc):
                if st_.pv is None:
                    st_.pv = [
                        psum.tile([128, 512], FP32, tag=f"pv{h}", name=f"pv{h}")
                        for h in range(HL)
                    ]
                p_t, off = st_.p_slots.pop(kc)
                first, last = kc == 0, kc == KT - 1
                gc = st_.b * KT + kc
                for h in range(HL):
                    nc.tensor.matmul(
                        st_.pv[h][0:65, :],
                        lhsT=v3[:, gc * HL + h, :],
                        rhs=p_t[:, off + h * 512 : off + (h + 1) * 512],
                        start=first,
                        stop=last,
                    )

            def finish_qt(st_):
                b, qt = st_.b, st_.qt
                # normalize-chain DMAs issue from the GpSimd queue (~10%
                # busy) instead of the congested Sync queue: they sit inline
                # with the chain's partition_broadcast and the collective,
                # skipping the Sync backlog of x-loads and stores
                for h in range(HL):
                    # single copy releases the PV PSUM bank; the rest of the
                    # normalize chain runs on SBUF off the fast path
                    o_c = norm.tile([65, 512], FP32, tag="oc", name="o_c")
                    nc.vector.tensor_copy(o_c[:], st_.pv[h][0:65, :])
                    # reciprocal across 128 partitions (not 512 sequential)
                    rs = norm.tile([128, 4], FP32, tag="rs", name="rs")
                    nc.gpsimd.dma_start(rs[:], o_c[64:65, :])
                    rr = norm.tile([128, 4], FP32, tag="rr", name="rr")
                    nc.vector.reciprocal(rr[:], rs[:])
                    rec = norm.tile([1, 512], FP32, tag="rec", name="rec")
                    nc.gpsimd.dma_start(rec[:], rr[:])
                    bc = norm.tile([64, 512], FP32, tag="bc", name="bc")
                    nc.gpsimd.partition_broadcast(bc[:], rec[:])
                    o_t = otp.tile([64, 512], BF16, tag="o", name="o_t")
                    nc.vector.tensor_mul(o_t[:], o_c[0:64, :], bc[:])
                    if b < B - 1:
                        nc.gpsimd.dma_start(
                            a2a_in[b][
                                2 * qt : 2 * qt + 2, h * 64 : (h + 1) * 64, :
                            ].rearrange("j p e -> p j e"),
                            o_t[:].rearrange("p (j e) -> p j e", j=2),
                        )
                    elif qt < 2:
                        j0 = (qt % 2) * 4
                        nc.gpsimd.dma_start(
                            a2a_in3[0][
                                j0 : j0 + 4, h * 64 : (h + 1) * 64, :
                            ].rearrange("j p e -> p j e"),
                            o_t[:].rearrange("p (j e) -> p j e", j=4),
                        )
                    else:
                        nc.gpsimd.dma_start(
                            a2a_in3[qt - 1][
                                :, h * 64 : (h + 1) * 64, :
                            ].rearrange("j p e -> p j e"),
                            o_t[:].rearrange("p (j e) -> p j e", j=8),
                        )

            # ---------------- score tiles ----------------
            def emit_stile(st_, kcs):
                b, qt = st_.b, st_.qt
                q_off = b * NTOK + qt * TN
                if len(kcs) == 2:
                    s_t = psum.tile([128, 2048], FP32, tag="q4", name="s_q")
                    p_t = pqp.tile([128, 2048], BF16, tag="p4", name="p_q")
                else:
                    s_t = psum.tile([128, 1024], FP32, tag="d2", name="s_d")
                    p_t = pdp.tile([128, 1024], BF16, tag="p2", name="p_d")
                width = 1024 * len(kcs)
                for ci, kc in enumerate(kcs):
                    koff = b * NTOK + kc * 128
                    for h in range(HL):
                        nc.tensor.matmul(
                            s_t[:, (ci * 2 + h) * 512 : (ci * 2 + h + 1) * 512],
                            lhsT=k_sb[h * 64 : (h + 1) * 64, koff : koff + 128],
                            rhs=q_sb[h * 64 : (h + 1) * 64, q_off : q_off + TN],
                            start=True,
                            stop=True,
                        )
                nc.scalar.activation(p_t[:, 0:width], s_t[:, 0:width], AF.Exp)
                for ci, kc in enumerate(kcs):
                    st_.p_slots[kc] = (p_t, ci * 1024)

            # ---------------- out projection ----------------
            oproj_state = {}

            def emit_osb_load(bsrc):
                o_sb = osbp.tile(
                    [128, N_CORES * TPB], BF16, tag="osb", name="o_sb"
                )
                nc.sync.dma_start(
                    o_sb[:].rearrange("p (i e) -> p i e", i=N_CORES),
                    a2a_out[bsrc][:, :, :].rearrange("i p e -> p i e"),
                )
                oproj_state["sb"] = o_sb

            def emit_outproj_m(bsrc, m):
                o_sb = oproj_state["sb"]
                o_ps = psum.tile([128, 1024], FP32, tag="d2", name="o_ps")
                for i in range(N_CORES):
                    for nh in range(2):
                        nc.tensor.matmul(
                            o_ps[:, nh * 512 : (nh + 1) * 512],
                            lhsT=o_sb[
                                :, i * TPB + m * 128 : i * TPB + (m + 1) * 128
                            ],
                            rhs=wo_sb[
                                :, i * D + nh * 512 : i * D + nh * 512 + 512
                            ],
                            start=(i == 0),
                            stop=(i == N_CORES - 1),
                        )
                out_t = fin.tile([128, D], FP32, tag="outt", name="out_t")
                nc.vector.tensor_add(out_t[:, :], o_ps[:, :], bias_sb[:, :])
                nc.sync.dma_start(
                    out[bsrc * TPB + m * 128 : bsrc * TPB + (m + 1) * 128, :],
                    out_t[:],
                )

            def emit_outproj3(pi, o_sbh, wtok, row0):
                # piece pi of batch 3: wtok tokens/core starting at out row0
                o_ps = psum.tile([128, 1024], FP32, tag="d2", name="o3_ps")
                for i in range(N_CORES):
                    for nh in range(2):
                        nc.tensor.matmul(
                            o_ps[0:wtok, nh * 512 : (nh + 1) * 512],
                            lhsT=o_sbh[:, i * wtok : (i + 1) * wtok],
                            rhs=wo_sb[
                                :, i * D + nh * 512 : i * D + nh * 512 + 512
                            ],
                            start=(i == 0),
                            stop=(i == N_CORES - 1),
                        )
                out_t = fin.tile([128, D], FP32, tag="outt", name="out_t")
                nc.vector.tensor_add(
                    out_t[0:wtok, :], o_ps[0:wtok, :], bias_sb[0:wtok, :]
                )
                nc.sync.dma_start(
                    out[row0 : row0 + wtok, :], out_t[0:wtok, :]
                )

            def emit_osbh_load(pi, wtok):
                o_sbh = osbp.tile(
                    [128, N_CORES * 128], BF16, tag="osbh", name=f"o_sbh{pi}"
                )
                nc.sync.dma_start(
                    o_sbh[:, 0 : N_CORES * wtok].rearrange(
                        "p (i e) -> p i e", i=N_CORES
                    ),
                    a2a_out3[pi][:, :, :].rearrange("i p e -> p i e"),
                )
                return o_sbh

            # ---------------- schedule ----------------
            def qt_stiles():
                # [Q(0,1) D(2) Q(3,4) D(5) Q(6,7) D(8) Q(9,10) D(11)
                #  Q(12,13) D(14) D(15)]
                s, kc = [], 0
                for _ in range(5):
                    s.append([kc, kc + 1])
                    kc += 2
                    s.append([kc])
                    kc += 1
                s.append([kc])
                return s

            pending = []  # (state, kcs) score tiles with PV flush deferred

            def note_stile(st_, kcs):
                # flush PV (three score-tiles behind) BEFORE the new stile's
                # S matmuls: the ready PV work then precedes the S quad in
                # the in-order PE FIFO, so a quad waiting on its ring's exp
                # no longer head-blocks work that could run
                while len(pending) > 2:
                    pst, pkcs = pending.pop(0)
                    for kc in pkcs:
                        flush_pv(pst, kc)
                emit_stile(st_, kcs)
                pending.append((st_, list(kcs)))

            def drain_pending():
                while pending:
                    pst, pkcs = pending.pop(0)
                    for kc in pkcs:
                        flush_pv(pst, kc)

            prev_qt_state = None

            # ---- batch 0 prologue: own QKV + qt0 interleaved ----
            emit_x_load(0)
            emit_x_load(1)
            st0 = QtState(0, 0)
            stiles0 = qt_stiles()
            g_next = 0
            for i in range(4):
                emit_qkv_tile(i)
                emit_x_load(i + 2)  # tiles 2..5 (tiles 4,5 belong to b1)
                if i == 0:
                    emit_wo_load()  # deferred so x0/w_sb loads go first
                avail = 4 * (i + 1)
                while g_next < len(stiles0) and all(
                    kc < avail for kc in stiles0[g_next]
                ):
                    note_stile(st0, stiles0[g_next])
                    g_next += 1
            emit_qkv_tile(4)  # b1 tile 0
            prev_qt_state = st0

            for b in range(B):
                for qt in range(4):
                    if b == 0 and qt == 0:
                        continue  # prologue handled it
                    st_ = QtState(b, qt)
                    # prefetch x for the QKV tile one slot ahead
                    nqt = 4 * (b + 1) + qt + 1
                    if nqt < 16 and nqt not in x_tiles:
                        emit_x_load(nqt)
                    for ti, kcs in enumerate(qt_stiles()):
                        # the previous qt's last PV flush happens at ti==2's
                        # note_stile (pending depth 3), so finish at ti==3
                        if ti == 3 and prev_qt_state is not None:
                            finish_qt(prev_qt_state)
                            pb, pq_ = prev_qt_state.b, prev_qt_state.qt
                            prev_qt_state = None
                            if pq_ == 3 and pb < B - 1:
                                emit_a2a(a2a_in[pb], a2a_out[pb])
                            elif pb == B - 1 and pq_ == 1:
                                emit_a2a(a2a_in3[0], a2a_out3[0])
                            elif pb == B - 1 and pq_ == 2:
                                emit_a2a(a2a_in3[1], a2a_out3[1])
                        note_stile(st_, kcs)
                        if ti == 2 and b < B - 1:
                            gt = 4 * (b + 1) + qt
                            if gt in x_tiles:
                                emit_qkv_tile(gt)
                        if b >= 1 and qt == 2 and ti == 6:
                            emit_osb_load(b - 1)
                        if b >= 1 and qt == 2 and ti == 8:
                            emit_outproj_m(b - 1, 0)
                        if b >= 1 and qt == 3 and ti == 4:
                            emit_outproj_m(b - 1, 1)
                    prev_qt_state = st_

            # ---- epilogue: last qt's PV + finish, tail pieces ----
            # piece-0/1 collectives completed during qt3: their out-proj PE
            # work runs under qt3's norm chain + the final collective,
            # keeping HAM warm through the tail
            drain_pending()
            o_sbh0 = emit_osbh_load(0, 128)
            o_sbh1 = emit_osbh_load(1, 64)
            finish_qt(prev_qt_state)  # qt3 -> a2a_in3[2]
            emit_outproj3(0, o_sbh0, 128, 768)
            emit_a2a(a2a_in3[2], a2a_out3[2])
            emit_outproj3(1, o_sbh1, 64, 896)
            o_sbh2 = emit_osbh_load(2, 64)
            emit_outproj3(2, o_sbh2, 64, 960)

    nc.compile()
    return nc


_NC_CACHE = None


def _get_nc():
    global _NC_CACHE
    if _NC_CACHE is None:
        _NC_CACHE = build_nc()
    return _NC_CACHE


def make_in_maps(x, w_qkv, w_out, b_out):
    x = np.asarray(x, dtype=np.float32)
    w_qkv = np.asarray(w_qkv, dtype=np.float32)
    w_out = np.asarray(w_out, dtype=np.float32)
    b_out = np.asarray(b_out, dtype=np.float32)

    xt_np = np.ascontiguousarray(x.reshape(T, D).T).astype(ml_dtypes.bfloat16)
    wo_np = np.ascontiguousarray(w_out.T).astype(ml_dtypes.bfloat16)
    b_np = np.ascontiguousarray(b_out.reshape(1, D))

    in_maps = []
    for c in range(N_CORES):
        rows = []
        for sec in range(3):  # q, k, v sections of w_qkv
            for hh in range(HL):
                h = HL * c + hh
                rows.append(w_qkv[sec * D + h * HD : sec * D + (h + 1) * HD, :])
        wt_np = np.ascontiguousarray(np.concatenate(rows, 0).T).astype(
            ml_dtypes.bfloat16
        )  # (1024, 384)
        in_maps.append({"xt": xt_np, "wt": wt_np, "wo": wo_np, "bias": b_np})
    return in_maps


def kernel(x, w_qkv, w_out, b_out, _trace=False, _tmpdir=None):
    in_maps = make_in_maps(x, w_qkv, w_out, b_out)
    nc = _get_nc()
    res = bass_utils.run_bass_kernel_spmd(
        nc, in_maps, core_ids=list(range(N_CORES)), trace=_trace, tmpdir=_tmpdir
    )
    # core j out rows:
    #   batches 0-2: r = b*256+u       -> token b*2048 + j*256 + u
    #   batch 3 piece0 (qt0+1): r = 768+u  (u<128) -> token 6144 + j*128 + u
    #   batch 3 piece1 (qt2):   r = 896+u  (u<64)  -> token 7168 + j*64 + u
    #   batch 3 piece2 (qt3):   r = 960+u  (u<64)  -> token 7680 + j*64 + u
    full = np.empty((T, D), np.float32)
    for j in range(N_CORES):
        o = np.asarray(res.results[j]["out"], dtype=np.float32)
        for b in range(B - 1):
            full[b * NTOK + j * TPB : b * NTOK + (j + 1) * TPB] = o[
                b * TPB : (b + 1) * TPB
            ]
        full[6144 + j * 128 : 6144 + (j + 1) * 128] = o[768:896]
        full[7168 + j * 64 : 7168 + (j + 1) * 64] = o[896:960]
        full[7680 + j * 64 : 7680 + (j + 1) * 64] = o[960:1024]
    kernel.last_result = res
    return full.reshape(B, NTOK, D)

